# Initial kernel scaffold
#
"""Deformable PS-ROI pooling on Trainium2 (Bass/Tile), SPMD over 8 cores.

Strategy: data-parallel over ROIs (64 rois/core), feature map replicated in
DRAM in channel-last layout so each bilinear corner is one contiguous 1 KiB
gather.  Per (roi, bin) the 4 samples x 4 corners = 16 gathered pixel vectors
land on 16 SBUF partitions (112 per 7-bin chunk); a block-diagonal 0/1 mask
matmul on the PE reduces them into the [49, 256] output, with all bilinear /
validity / 1-over-count factors pre-folded into a per-partition scalar weight.
"""

import numpy as np

import concourse.bass as bass
import concourse.bacc as bacc
import concourse.mybir as mybir
from concourse import tile
from concourse.bass import IndirectOffsetOnAxis
from concourse.bass_utils import run_bass_kernel_spmd

F32 = mybir.dt.float32
F32R = mybir.dt.float32r
I32 = mybir.dt.int32
OP = mybir.AluOpType

N_CORES = 8
R = 64                  # rois per core
P = 7                   # pooled output size
NB = P * P              # 49 bins
CH = 256                # channels
H = W = 128             # feature map spatial
B = 2                   # batch
NPX = B * H * W         # 32768 flat pixels
TPB = 16                # terms (sample x corner) per bin
T = NB * TPB            # 784 terms per roi
KC = 112                # terms per K-chunk = 7 bins
NCH = 7                 # chunks per roi
G = 1                   # rois per gather group (Q7 idx scratch caps ~1024 descs)
SCALE = 0.0625
TRANS_STD = 0.1


def _floor(nc, pool, x, name):
    """floor(x) robust to convert rounding mode: returns (floor_f32, frac)."""
    xi = pool.tile([R, x.shape[1]], I32, tag=name + "_i")
    nc.vector.tensor_copy(xi[:, :], x)
    xf = pool.tile([R, x.shape[1]], F32, tag=name + "_f")
    nc.vector.tensor_copy(xf[:, :], xi[:, :])
    d = pool.tile([R, x.shape[1]], F32, tag=name + "_d")
    nc.vector.tensor_tensor(d[:, :], x, xf[:, :], OP.subtract)
    neg = pool.tile([R, x.shape[1]], F32, tag=name + "_n")
    nc.vector.tensor_scalar(neg[:, :], d[:, :], 0.0, None, OP.is_lt)
    fl = pool.tile([R, x.shape[1]], F32, tag=name + "_fl")
    nc.vector.tensor_tensor(fl[:, :], xf[:, :], neg[:, :], OP.subtract)
    fr = pool.tile([R, x.shape[1]], F32, tag=name + "_fr")
    nc.vector.tensor_tensor(fr[:, :], d[:, :], neg[:, :], OP.add)
    return fl[:, :], fr[:, :]


def build_program(reps: int = 1):
    nc = bacc.Bacc("TRN2", target_bir_lowering=False, debug=False, num_swdge_queues=4)
    nc.dynamic_dma_scratch_size = 2 ** 16

    data = nc.dram_tensor("data_t", [NPX, CH], F32, kind="ExternalInput")
    rois_d = nc.dram_tensor("rois", [R, 5], F32, kind="ExternalInput")
    off_d = nc.dram_tensor("offs", [R, 2 * NB], F32, kind="ExternalInput")
    iopw_d = nc.dram_tensor("iota_pw", [R, NB], F32, kind="ExternalInput")
    ioph_d = nc.dram_tensor("iota_ph", [R, NB], F32, kind="ExternalInput")
    iden_d = nc.dram_tensor("identity", [R, R], F32, kind="ExternalInput")
    cmsk_d = nc.dram_tensor("cmasks", [128, NCH * NB], F32, kind="ExternalInput")
    out_d = nc.dram_tensor("out", [R, NB * CH], F32, kind="ExternalOutput")

    with tile.TileContext(nc) as tc:
        with (
            tc.tile_pool(name="const", bufs=1) as cst,
            tc.tile_pool(name="work", bufs=1) as wk,
            tc.tile_pool(name="gp", bufs=10) as gp,
            tc.tile_pool(name="gwp", bufs=12) as gwp,
            tc.tile_pool(name="obp", bufs=8) as obp,
            tc.tile_pool(name="psp", bufs=6, space="PSUM") as psp,
            tc.tile_pool(name="pst", bufs=2, space="PSUM") as pst,
        ):
            # ---- load inputs / constants to SBUF ----
            rois = cst.tile([R, 5], F32)
            nc.sync.dma_start(rois[:, :], rois_d.ap())
            off = cst.tile([R, 2 * NB], F32)
            nc.sync.dma_start(off[:, :], off_d.ap())
            iopw = cst.tile([R, NB], F32)
            nc.sync.dma_start(iopw[:, :], iopw_d.ap())
            ioph = cst.tile([R, NB], F32)
            nc.sync.dma_start(ioph[:, :], ioph_d.ap())
            iden = cst.tile([R, R], F32)
            nc.sync.dma_start(iden[:, :], iden_d.ap())
            cmsk = cst.tile([128, NCH * NB], F32)
            nc.sync.dma_start(cmsk[:, :], cmsk_d.ap())

            # ---- phase A: per-roi coordinate math, roi on partition ----
            # round(rois[:,1:5]) = floor(x + 0.5)
            rr = wk.tile([R, 4], F32)
            nc.vector.tensor_scalar(rr[:, :], rois[:, 1:5], 0.5, None, OP.add)
            rnd, _ = _floor(nc, wk, rr[:, :], "rnd")

            # start/end in feature coords
            swsh = wk.tile([R, 2], F32)
            nc.vector.tensor_scalar(swsh[:, :], rnd[:, 0:2], SCALE, -0.5, OP.mult, OP.add)
            eweh = wk.tile([R, 2], F32)
            nc.vector.tensor_scalar(
                eweh[:, :], rnd[:, 2:4], SCALE, SCALE - 0.5, OP.mult, OP.add
            )
            rwh0 = wk.tile([R, 2], F32)
            nc.vector.tensor_tensor(rwh0[:, :], eweh[:, :], swsh[:, :], OP.subtract)
            rwh = wk.tile([R, 2], F32)
            nc.vector.tensor_scalar(rwh[:, :], rwh0[:, :], 0.1, None, OP.max)
            bwh = wk.tile([R, 2], F32)
            nc.vector.tensor_scalar(bwh[:, :], rwh[:, :], 1.0 / P, None, OP.mult)
            swh = wk.tile([R, 2], F32)
            nc.vector.tensor_scalar(swh[:, :], bwh[:, :], 0.5, None, OP.mult)
            rwh01 = wk.tile([R, 2], F32)
            nc.vector.tensor_scalar(rwh01[:, :], rwh[:, :], TRANS_STD, None, OP.mult)
            ybase = wk.tile([R, 1], F32)
            nc.vector.tensor_scalar(ybase[:, :], rois[:, 0:1], float(H * W), None, OP.mult)

            # bin starts, shifted by learned offsets: [R, 49]
            def bin_start(iota, bcol, scol, tview, r01col, name):
                t0 = wk.tile([R, NB], F32, tag=name + "0")
                nc.vector.tensor_scalar(t0[:, :], iota, bcol, None, OP.mult)
                t1 = wk.tile([R, NB], F32, tag=name + "1")
                nc.vector.scalar_tensor_tensor(
                    t1[:, :], tview, r01col, t0[:, :], OP.mult, OP.add
                )
                t2 = wk.tile([R, NB], F32, tag=name + "2")
                nc.vector.tensor_scalar(t2[:, :], t1[:, :], scol, None, OP.add)
                return t2

            wstart = bin_start(
                iopw[:, :], bwh[:, 0:1], swsh[:, 0:1], off[:, 0:NB],
                rwh01[:, 0:1], "ws",
            )
            hstart = bin_start(
                ioph[:, :], bwh[:, 1:2], swsh[:, 1:2], off[:, NB : 2 * NB],
                rwh01[:, 1:2], "hs",
            )

            # sample positions [R, 98] = (bin, s)
            def samples(start, subcol, name):
                s2 = wk.tile([R, 2 * NB], F32, tag=name)
                v = s2[:, :].rearrange("p (b s) -> p b s", s=2)
                su = start[:, :].rearrange("p b -> p b", ).unsqueeze(2)
                nc.vector.tensor_copy(v[:, :, 0:1], su)
                nc.vector.tensor_scalar(v[:, :, 1:2], su, subcol, None, OP.add)
                return s2

            X2 = samples(wstart, swh[:, 0:1], "X2")
            Y2 = samples(hstart, swh[:, 1:2], "Y2")

            # per-axis: validity, clip, floor/frac, weight pairs, index pairs
            def axis_side(S2, lim, name):
                # valid = (S2 >= -0.5) & (S2 <= lim + 0.5)
                va = wk.tile([R, 2 * NB], F32, tag=name + "va")
                nc.vector.tensor_scalar(va[:, :], S2[:, :], -0.5, None, OP.is_ge)
                vv = wk.tile([R, 2 * NB], F32, tag=name + "vv")
                nc.vector.scalar_tensor_tensor(
                    vv[:, :], S2[:, :], lim + 0.5, va[:, :], OP.is_le, OP.mult
                )
                cl = wk.tile([R, 2 * NB], F32, tag=name + "cl")
                nc.vector.tensor_scalar(cl[:, :], S2[:, :], 0.0, lim, OP.max, OP.min)
                flo, fra = _floor(nc, wk, cl[:, :], name + "fl")
                # count over the 2 samples, per bin -> reciprocal (exact: 1 or .5)
                cnt = wk.tile([R, NB], F32, tag=name + "ct")
                vvv = vv[:, :].rearrange("p (b s) -> p b s", s=2)
                nc.vector.tensor_tensor(
                    cnt[:, :].unsqueeze(2),
                    vvv[:, :, 0:1], vvv[:, :, 1:2], OP.add,
                )
                eq2 = wk.tile([R, NB], F32, tag=name + "e2")
                nc.vector.tensor_scalar(eq2[:, :], cnt[:, :], 2.0, None, OP.is_equal)
                rc = wk.tile([R, NB], F32, tag=name + "rc")
                nc.vector.tensor_scalar(rc[:, :], eq2[:, :], -0.5, 1.0, OP.mult, OP.add)
                # weight pair: w0 = v*(1-f)*rc, w1 = v*f*rc  [R, 196] = (bin, s, c)
                rcb = rc[:, :].unsqueeze(2).broadcast_to([R, NB, 2])
                vr = wk.tile([R, 2 * NB], F32, tag=name + "vr")
                nc.vector.tensor_tensor(
                    vr[:, :].rearrange("p (b s) -> p b s", s=2), vvv, rcb, OP.mult
                )
                w1 = wk.tile([R, 2 * NB], F32, tag=name + "w1")
                nc.vector.tensor_tensor(w1[:, :], vr[:, :], fra, OP.mult)
                w0 = wk.tile([R, 2 * NB], F32, tag=name + "w0")
                nc.vector.tensor_tensor(w0[:, :], vr[:, :], w1[:, :], OP.subtract)
                W4 = wk.tile([R, 4 * NB], F32, tag=name + "W4")
                W4v = W4[:, :].rearrange("p (b s c) -> p b s c", s=2, c=2)
                w0v = w0[:, :].rearrange("p (b s) -> p b s", s=2).unsqueeze(3)
                w1v = w1[:, :].rearrange("p (b s) -> p b s", s=2).unsqueeze(3)
                nc.vector.tensor_copy(W4v[:, :, :, 0:1], w0v)
                nc.vector.tensor_copy(W4v[:, :, :, 1:2], w1v)
                # index pair: i0 = floor, i1 = min(floor+1, lim)
                I4 = wk.tile([R, 4 * NB], F32, tag=name + "I4")
                I4v = I4[:, :].rearrange("p (b s c) -> p b s c", s=2, c=2)
                flv = flo.rearrange("p (b s) -> p b s", s=2).unsqueeze(3)
                nc.vector.tensor_copy(I4v[:, :, :, 0:1], flv)
                nc.vector.tensor_scalar(I4v[:, :, :, 1:2], flv, 1.0, lim, OP.add, OP.min)
                return W4, I4

            WX4, XI4 = axis_side(X2, float(W - 1), "x")
            WY4, YI4 = axis_side(Y2, float(H - 1), "y")

            # y-side indices -> flat row base: b*H*W + y*W
            YIr = wk.tile([R, 4 * NB], F32)
            nc.vector.tensor_scalar(
                YIr[:, :], YI4[:, :], float(W), ybase[:, :], OP.mult, OP.add
            )

            # expand to full terms [R, 784] = (bin, sh, cy, sw, cx)
            Wt = wk.tile([R, T], F32)
            Wtv = Wt[:, :].rearrange("p (b h y s x) -> p b h y s x", h=2, y=2, s=2, x=2)
            IDX = wk.tile([R, T], F32)
            IDXv = IDX[:, :].rearrange(
                "p (b h y s x) -> p b h y s x", h=2, y=2, s=2, x=2
            )
            WY4v = WY4[:, :].rearrange("p (b h y) -> p b h y", h=2, y=2).unsqueeze(4).unsqueeze(5)
            YIrv = YIr[:, :].rearrange("p (b h y) -> p b h y", h=2, y=2).unsqueeze(4).unsqueeze(5)
            for k in range(4):
                s, x = k >> 1, k & 1
                nc.vector.tensor_copy(Wtv[:, :, :, :, s : s + 1, x : x + 1], WY4v)
                nc.vector.tensor_copy(IDXv[:, :, :, :, s : s + 1, x : x + 1], YIrv)
            WX4v = WX4[:, :].rearrange("p (b s x) -> p b s x", s=2, x=2).unsqueeze(2).unsqueeze(3)
            XI4v = XI4[:, :].rearrange("p (b s x) -> p b s x", s=2, x=2).unsqueeze(2).unsqueeze(3)
            for j in range(4):
                h, y = j >> 1, j & 1
                dstW = Wtv[:, :, h : h + 1, y : y + 1, :, :]
                dstI = IDXv[:, :, h : h + 1, y : y + 1, :, :]
                nc.vector.tensor_tensor(dstW, dstW, WX4v, OP.mult)
                nc.vector.tensor_tensor(dstI, dstI, XI4v, OP.add)
            # ---- phase B: transpose weights to [128, (n,c)]; build int16
            # gather indices in dma_gather's 16-lane-wrapped layout.
            # Descriptor i = m*16 + l reads IDXG[l, m]; lands at dest
            # partition i%128, col i//128.  With idx col m = n*56 + q*8 + rr
            # (rr<7 real, rr=7 pad), term (roi n, chunk q, bin-in-chunk rr,
            # corner k=l) lands at partition rr*16+k, dest col n*7+q.
            WT = wk.tile([128, R * NCH], F32)
            nc.vector.memset(WT[:, :], 0.0)
            WTv = WT[:, :].rearrange("p (n c) -> p c n", c=NCH)
            for c in range(NCH):
                kc = 128 if c < 6 else 16
                psA = pst.tile([kc, R], F32, tag="pstr")
                nc.tensor.transpose(
                    psA[:, :], Wt[:, c * 128 : c * 128 + kc], iden[:, :]
                )
                nc.vector.tensor_copy(
                    WTv[0:kc, c : c + 1, :], psA[:, :].unsqueeze(1)
                )
            IDXG = wk.tile([128, R * NB], mybir.dt.int16)
            nc.vector.memset(IDXG[:, :], 0)
            IDXGv = IDXG[:, :].rearrange("p (n m) -> p n m", m=NB)
            for b in range(NB):
                psB = pst.tile([16, R], F32, tag="pstr")
                nc.tensor.transpose(
                    psB[:, :], IDX[:, b * TPB : (b + 1) * TPB], iden[:, :]
                )
                nc.vector.tensor_copy(
                    IDXGv[0:16, :, b : b + 1], psB[:, :].unsqueeze(2)
                )
            # Q7 tx/rx cpus each read their own 16-partition window of the
            # index tensor -> replicate lane group 0 across all 8 groups.
            for grp in range(1, 8):
                nc.sync.dma_start(
                    IDXG[16 * grp : 16 * (grp + 1), :], IDXG[0:16, :]
                )

            # ---- phase C: gather + weighted reduce ----
            NI = T  # 784 descriptors per roi, no padding
            out_r = out_d.ap().rearrange("r (b c) -> r b c", c=CH)
            from contextlib import nullcontext
            loop_cm = tc.For_i(0, reps, 1) if reps > 1 else nullcontext()
            with loop_cm:
              for g in range(R // G):
                  gt = gp.tile([128, NCH * CH], F32)
                  # col 6 rows 16-127 are never gathered; clear for finite 0s
                  nc.vector.memset(gt[:, 6 * CH : 7 * CH], 0.0)
                  dest = gt[:, :].rearrange("p (j f) -> p j f", f=CH)
                  nc.gpsimd.dma_gather(
                      dest,
                      data.ap(),
                      IDXG[:, g * NB : (g + 1) * NB],
                      NI,
                      NI,
                      CH,
                      queue_num=g % 4,
                  )
                  for nl in range(G):
                      n = g * G + nl
                      # weighted mask for all 7 chunks of this roi in one op:
                      # wm[p, c, j] = cmask[p, c, j] * WT[p, n*7+c]
                      wm = gwp.tile([128, NCH * NB], F32)
                      wtb = (
                          WT[:, n * NCH : (n + 1) * NCH]
                          .unsqueeze(2)
                          .broadcast_to([128, NCH, NB])
                      )
                      nc.any.tensor_tensor(
                          wm[:, :].rearrange("p (c j) -> p c j", j=NB),
                          cmsk[:, :].rearrange("p (c j) -> p c j", j=NB),
                          wtb,
                          OP.mult,
                      )
                      ps = psp.tile([NB, CH], F32)
                      for c in range(NCH):
                          gv = gt[:, (nl * NCH + c) * CH : (nl * NCH + c + 1) * CH]
                          nc.tensor.matmul(
                              ps[:, :],
                              wm[:, c * NB : (c + 1) * NB],
                              gv,
                              start=(c == 0),
                              stop=(c == NCH - 1),
                          )
                      ob = obp.tile([NB, CH], F32)
                      nc.vector.tensor_copy(ob[:, :], ps[:, :])
                      nc.sync.dma_start(out_r[n : n + 1, :, :], ob[:, :])

    nc.finalize()
    return nc


def host_constants():
    iopw = np.tile((np.arange(NB) % P).astype(np.float32), (R, 1))
    ioph = np.tile((np.arange(NB) // P).astype(np.float32), (R, 1))
    iden = np.eye(R, dtype=np.float32)
    cm = np.zeros((128, NCH * NB), dtype=np.float32)
    for j in range(6):
        for p in range(128):
            cm[p, j * NB + 8 * j + p // TPB] = 1.0
    for p in range(TPB):
        cm[p, 6 * NB + 48] = 1.0
    return {"iota_pw": iopw, "iota_ph": ioph, "identity": iden, "cmasks": cm}


_cache = {}


def _program():
    if "nc" not in _cache:
        _cache["nc"] = build_program()
    return _cache["nc"]


def run(data, rois, offset, **spmd_kwargs):
    data = np.asarray(data, dtype=np.float32)
    rois = np.asarray(rois, dtype=np.float32)
    offset = np.asarray(offset, dtype=np.float32)
    n_rois = rois.shape[0]
    data_t = np.ascontiguousarray(data.transpose(0, 2, 3, 1)).reshape(NPX, CH)
    consts = host_constants()
    in_maps = []
    for c in range(N_CORES):
        sl = slice(c * R, (c + 1) * R)
        m = {
            "data_t": data_t,
            "rois": rois[sl],
            "offs": offset[sl].reshape(R, 2 * NB),
        }
        m.update(consts)
        in_maps.append(m)
    res = run_bass_kernel_spmd(
        _program(), in_maps, core_ids=list(range(N_CORES)), **spmd_kwargs
    )
    outs = np.concatenate([res.results[c]["out"] for c in range(N_CORES)], axis=0)
    out = outs.reshape(n_rois, NB, CH).transpose(0, 2, 1).reshape(n_rois, CH, P, P)
    return np.ascontiguousarray(out), res


def kernel(data, rois, offset):
    out, _ = run(data, rois, offset)
    return out



# revision 1
# speedup vs baseline: 5.9520x; 5.9520x over previous
"""Deformable PS-ROI pooling on Trainium2 (Bass/Tile), SPMD over 8 cores.

Strategy: data-parallel over ROIs (64 rois/core), feature map replicated in
DRAM in channel-last layout so each bilinear corner is one contiguous 1 KiB
gather.  Per (roi, bin) the 4 samples x 4 corners = 16 gathered pixel vectors
land on 16 SBUF partitions (112 per 7-bin chunk); a block-diagonal 0/1 mask
matmul on the PE reduces them into the [49, 256] output, with all bilinear /
validity / 1-over-count factors pre-folded into a per-partition scalar weight.
"""

import numpy as np

import concourse.bass as bass
import concourse.bacc as bacc
import concourse.mybir as mybir
from concourse import tile
from concourse.bass import IndirectOffsetOnAxis
from concourse.bass_utils import run_bass_kernel_spmd

F32 = mybir.dt.float32
F32R = mybir.dt.float32r
I32 = mybir.dt.int32
OP = mybir.AluOpType

N_CORES = 8
R = 64                  # rois per core
P = 7                   # pooled output size
NB = P * P              # 49 bins
CH = 256                # channels
H = W = 128             # feature map spatial
B = 2                   # batch
NPX = B * H * W         # 32768 flat pixels
TPB = 16                # terms (sample x corner) per bin
T = NB * TPB            # 784 terms per roi
KC = 112                # terms per K-chunk = 7 bins
NCH = 7                 # chunks per roi
G = 1                   # rois per gather group (Q7 idx scratch caps ~1024 descs)
SCALE = 0.0625
TRANS_STD = 0.1


def _floor(nc, pool, x, name):
    """floor(x) robust to convert rounding mode: returns (floor_f32, frac)."""
    xi = pool.tile([R, x.shape[1]], I32, tag=name + "_i")
    nc.vector.tensor_copy(xi[:, :], x)
    xf = pool.tile([R, x.shape[1]], F32, tag=name + "_f")
    nc.vector.tensor_copy(xf[:, :], xi[:, :])
    d = pool.tile([R, x.shape[1]], F32, tag=name + "_d")
    nc.vector.tensor_tensor(d[:, :], x, xf[:, :], OP.subtract)
    neg = pool.tile([R, x.shape[1]], F32, tag=name + "_n")
    nc.vector.tensor_scalar(neg[:, :], d[:, :], 0.0, None, OP.is_lt)
    fl = pool.tile([R, x.shape[1]], F32, tag=name + "_fl")
    nc.vector.tensor_tensor(fl[:, :], xf[:, :], neg[:, :], OP.subtract)
    fr = pool.tile([R, x.shape[1]], F32, tag=name + "_fr")
    nc.vector.tensor_tensor(fr[:, :], d[:, :], neg[:, :], OP.add)
    return fl[:, :], fr[:, :]


def build_program(reps: int = 1):
    nc = bacc.Bacc("TRN2", target_bir_lowering=False, debug=False, num_swdge_queues=4)
    nc.dynamic_dma_scratch_size = 2 ** 16

    data = nc.dram_tensor("data_t", [NPX, CH], F32, kind="ExternalInput")
    rois_d = nc.dram_tensor("rois", [R, 5], F32, kind="ExternalInput")
    off_d = nc.dram_tensor("offs", [R, 2 * NB], F32, kind="ExternalInput")
    iopw_d = nc.dram_tensor("iota_pw", [R, NB], F32, kind="ExternalInput")
    ioph_d = nc.dram_tensor("iota_ph", [R, NB], F32, kind="ExternalInput")
    iden_d = nc.dram_tensor("identity", [R, R], F32, kind="ExternalInput")
    cmsk_d = nc.dram_tensor("cmasks", [128, NCH * NB], F32, kind="ExternalInput")
    out_d = nc.dram_tensor("out", [R, NB * CH], F32, kind="ExternalOutput")

    with tile.TileContext(nc) as tc:
        with (
            tc.tile_pool(name="const", bufs=1) as cst,
            tc.tile_pool(name="work", bufs=1) as wk,
            tc.tile_pool(name="gp", bufs=10) as gp,
            tc.tile_pool(name="gwp", bufs=12) as gwp,
            tc.tile_pool(name="obp", bufs=8) as obp,
            tc.tile_pool(name="psp", bufs=6, space="PSUM") as psp,
            tc.tile_pool(name="pst", bufs=2, space="PSUM") as pst,
        ):
            # ---- load inputs / constants to SBUF ----
            rois = cst.tile([R, 5], F32)
            nc.sync.dma_start(rois[:, :], rois_d.ap())
            off = cst.tile([R, 2 * NB], F32)
            nc.sync.dma_start(off[:, :], off_d.ap())
            iopw = cst.tile([R, NB], F32)
            nc.sync.dma_start(iopw[:, :], iopw_d.ap())
            ioph = cst.tile([R, NB], F32)
            nc.sync.dma_start(ioph[:, :], ioph_d.ap())
            iden = cst.tile([R, R], F32)
            nc.sync.dma_start(iden[:, :], iden_d.ap())
            cmsk = cst.tile([128, NCH * NB], F32)
            nc.sync.dma_start(cmsk[:, :], cmsk_d.ap())

            # ---- phase A: per-roi coordinate math, roi on partition ----
            # round(rois[:,1:5]) = floor(x + 0.5)
            rr = wk.tile([R, 4], F32)
            nc.vector.tensor_scalar(rr[:, :], rois[:, 1:5], 0.5, None, OP.add)
            rnd, _ = _floor(nc, wk, rr[:, :], "rnd")

            # start/end in feature coords
            swsh = wk.tile([R, 2], F32)
            nc.vector.tensor_scalar(swsh[:, :], rnd[:, 0:2], SCALE, -0.5, OP.mult, OP.add)
            eweh = wk.tile([R, 2], F32)
            nc.vector.tensor_scalar(
                eweh[:, :], rnd[:, 2:4], SCALE, SCALE - 0.5, OP.mult, OP.add
            )
            rwh0 = wk.tile([R, 2], F32)
            nc.vector.tensor_tensor(rwh0[:, :], eweh[:, :], swsh[:, :], OP.subtract)
            rwh = wk.tile([R, 2], F32)
            nc.vector.tensor_scalar(rwh[:, :], rwh0[:, :], 0.1, None, OP.max)
            bwh = wk.tile([R, 2], F32)
            nc.vector.tensor_scalar(bwh[:, :], rwh[:, :], 1.0 / P, None, OP.mult)
            swh = wk.tile([R, 2], F32)
            nc.vector.tensor_scalar(swh[:, :], bwh[:, :], 0.5, None, OP.mult)
            rwh01 = wk.tile([R, 2], F32)
            nc.vector.tensor_scalar(rwh01[:, :], rwh[:, :], TRANS_STD, None, OP.mult)
            ybase = wk.tile([R, 1], F32)
            nc.vector.tensor_scalar(ybase[:, :], rois[:, 0:1], float(H * W), None, OP.mult)

            # bin starts, shifted by learned offsets: [R, 49]
            def bin_start(iota, bcol, scol, tview, r01col, name):
                t0 = wk.tile([R, NB], F32, tag=name + "0")
                nc.vector.tensor_scalar(t0[:, :], iota, bcol, None, OP.mult)
                t1 = wk.tile([R, NB], F32, tag=name + "1")
                nc.vector.scalar_tensor_tensor(
                    t1[:, :], tview, r01col, t0[:, :], OP.mult, OP.add
                )
                t2 = wk.tile([R, NB], F32, tag=name + "2")
                nc.vector.tensor_scalar(t2[:, :], t1[:, :], scol, None, OP.add)
                return t2

            wstart = bin_start(
                iopw[:, :], bwh[:, 0:1], swsh[:, 0:1], off[:, 0:NB],
                rwh01[:, 0:1], "ws",
            )
            hstart = bin_start(
                ioph[:, :], bwh[:, 1:2], swsh[:, 1:2], off[:, NB : 2 * NB],
                rwh01[:, 1:2], "hs",
            )

            # sample positions [R, 98] = (bin, s)
            def samples(start, subcol, name):
                s2 = wk.tile([R, 2 * NB], F32, tag=name)
                v = s2[:, :].rearrange("p (b s) -> p b s", s=2)
                su = start[:, :].rearrange("p b -> p b", ).unsqueeze(2)
                nc.vector.tensor_copy(v[:, :, 0:1], su)
                nc.vector.tensor_scalar(v[:, :, 1:2], su, subcol, None, OP.add)
                return s2

            X2 = samples(wstart, swh[:, 0:1], "X2")
            Y2 = samples(hstart, swh[:, 1:2], "Y2")

            # per-axis: validity, clip, floor/frac, weight pairs, index pairs
            def axis_side(S2, lim, name):
                # valid = (S2 >= -0.5) & (S2 <= lim + 0.5)
                va = wk.tile([R, 2 * NB], F32, tag=name + "va")
                nc.vector.tensor_scalar(va[:, :], S2[:, :], -0.5, None, OP.is_ge)
                vv = wk.tile([R, 2 * NB], F32, tag=name + "vv")
                nc.vector.scalar_tensor_tensor(
                    vv[:, :], S2[:, :], lim + 0.5, va[:, :], OP.is_le, OP.mult
                )
                cl = wk.tile([R, 2 * NB], F32, tag=name + "cl")
                nc.vector.tensor_scalar(cl[:, :], S2[:, :], 0.0, lim, OP.max, OP.min)
                flo, fra = _floor(nc, wk, cl[:, :], name + "fl")
                # count over the 2 samples, per bin -> reciprocal (exact: 1 or .5)
                cnt = wk.tile([R, NB], F32, tag=name + "ct")
                vvv = vv[:, :].rearrange("p (b s) -> p b s", s=2)
                nc.vector.tensor_tensor(
                    cnt[:, :].unsqueeze(2),
                    vvv[:, :, 0:1], vvv[:, :, 1:2], OP.add,
                )
                eq2 = wk.tile([R, NB], F32, tag=name + "e2")
                nc.vector.tensor_scalar(eq2[:, :], cnt[:, :], 2.0, None, OP.is_equal)
                rc = wk.tile([R, NB], F32, tag=name + "rc")
                nc.vector.tensor_scalar(rc[:, :], eq2[:, :], -0.5, 1.0, OP.mult, OP.add)
                # weight pair: w0 = v*(1-f)*rc, w1 = v*f*rc  [R, 196] = (bin, s, c)
                rcb = rc[:, :].unsqueeze(2).broadcast_to([R, NB, 2])
                vr = wk.tile([R, 2 * NB], F32, tag=name + "vr")
                nc.vector.tensor_tensor(
                    vr[:, :].rearrange("p (b s) -> p b s", s=2), vvv, rcb, OP.mult
                )
                w1 = wk.tile([R, 2 * NB], F32, tag=name + "w1")
                nc.vector.tensor_tensor(w1[:, :], vr[:, :], fra, OP.mult)
                w0 = wk.tile([R, 2 * NB], F32, tag=name + "w0")
                nc.vector.tensor_tensor(w0[:, :], vr[:, :], w1[:, :], OP.subtract)
                W4 = wk.tile([R, 4 * NB], F32, tag=name + "W4")
                W4v = W4[:, :].rearrange("p (b s c) -> p b s c", s=2, c=2)
                w0v = w0[:, :].rearrange("p (b s) -> p b s", s=2).unsqueeze(3)
                w1v = w1[:, :].rearrange("p (b s) -> p b s", s=2).unsqueeze(3)
                nc.vector.tensor_copy(W4v[:, :, :, 0:1], w0v)
                nc.vector.tensor_copy(W4v[:, :, :, 1:2], w1v)
                # index pair: i0 = floor, i1 = min(floor+1, lim)
                I4 = wk.tile([R, 4 * NB], F32, tag=name + "I4")
                I4v = I4[:, :].rearrange("p (b s c) -> p b s c", s=2, c=2)
                flv = flo.rearrange("p (b s) -> p b s", s=2).unsqueeze(3)
                nc.vector.tensor_copy(I4v[:, :, :, 0:1], flv)
                nc.vector.tensor_scalar(I4v[:, :, :, 1:2], flv, 1.0, lim, OP.add, OP.min)
                return W4, I4

            WX4, XI4 = axis_side(X2, float(W - 1), "x")
            WY4, YI4 = axis_side(Y2, float(H - 1), "y")

            # y-side indices -> flat row base: b*H*W + y*W
            YIr = wk.tile([R, 4 * NB], F32)
            nc.vector.tensor_scalar(
                YIr[:, :], YI4[:, :], float(W), ybase[:, :], OP.mult, OP.add
            )

            # expand to full terms [R, 784] = (bin, sh, cy, sw, cx)
            Wt = wk.tile([R, T], F32)
            Wtv = Wt[:, :].rearrange("p (b h y s x) -> p b h y s x", h=2, y=2, s=2, x=2)
            IDX = wk.tile([R, T], F32)
            IDXv = IDX[:, :].rearrange(
                "p (b h y s x) -> p b h y s x", h=2, y=2, s=2, x=2
            )
            WY4v = WY4[:, :].rearrange("p (b h y) -> p b h y", h=2, y=2).unsqueeze(4).unsqueeze(5)
            YIrv = YIr[:, :].rearrange("p (b h y) -> p b h y", h=2, y=2).unsqueeze(4).unsqueeze(5)
            for k in range(4):
                s, x = k >> 1, k & 1
                nc.vector.tensor_copy(Wtv[:, :, :, :, s : s + 1, x : x + 1], WY4v)
                nc.vector.tensor_copy(IDXv[:, :, :, :, s : s + 1, x : x + 1], YIrv)
            WX4v = WX4[:, :].rearrange("p (b s x) -> p b s x", s=2, x=2).unsqueeze(2).unsqueeze(3)
            XI4v = XI4[:, :].rearrange("p (b s x) -> p b s x", s=2, x=2).unsqueeze(2).unsqueeze(3)
            for j in range(4):
                h, y = j >> 1, j & 1
                dstW = Wtv[:, :, h : h + 1, y : y + 1, :, :]
                dstI = IDXv[:, :, h : h + 1, y : y + 1, :, :]
                nc.vector.tensor_tensor(dstW, dstW, WX4v, OP.mult)
                nc.vector.tensor_tensor(dstI, dstI, XI4v, OP.add)
            # ---- phase B: transpose weights to [128, (n,c)]; build int16
            # gather indices in dma_gather's 16-lane-wrapped layout.
            # Descriptor i = m*16 + l reads IDXG[l, m]; lands at dest
            # partition i%128, col i//128.  With idx col m = n*56 + q*8 + rr
            # (rr<7 real, rr=7 pad), term (roi n, chunk q, bin-in-chunk rr,
            # corner k=l) lands at partition rr*16+k, dest col n*7+q.
            WT = wk.tile([128, R * NCH], F32)
            nc.vector.memset(WT[:, :], 0.0)
            WTv = WT[:, :].rearrange("p (n c) -> p c n", c=NCH)
            for c in range(NCH):
                kc = 128 if c < 6 else 16
                psA = pst.tile([kc, R], F32, tag="pstr")
                nc.tensor.transpose(
                    psA[:, :], Wt[:, c * 128 : c * 128 + kc], iden[:, :]
                )
                nc.vector.tensor_copy(
                    WTv[0:kc, c : c + 1, :], psA[:, :].unsqueeze(1)
                )
            IDXG = wk.tile([128, R * NB], mybir.dt.int16)
            nc.vector.memset(IDXG[:, :], 0)
            IDXGv = IDXG[:, :].rearrange("p (n m) -> p n m", m=NB)
            for b in range(NB):
                psB = pst.tile([16, R], F32, tag="pstr")
                nc.tensor.transpose(
                    psB[:, :], IDX[:, b * TPB : (b + 1) * TPB], iden[:, :]
                )
                nc.vector.tensor_copy(
                    IDXGv[0:16, :, b : b + 1], psB[:, :].unsqueeze(2)
                )
            # Q7 tx/rx cpus each read their own 16-partition window of the
            # index tensor -> replicate lane group 0 across all 8 groups.
            for grp in range(1, 8):
                nc.sync.dma_start(
                    IDXG[16 * grp : 16 * (grp + 1), :], IDXG[0:16, :]
                )

            # ---- phase C: gather + weighted reduce ----
            NI = T  # 784 descriptors per roi, no padding
            out_r = out_d.ap().rearrange("r (b c) -> r b c", c=CH)
            from contextlib import nullcontext
            loop_cm = tc.For_i(0, reps, 1) if reps > 1 else nullcontext()
            with loop_cm:
              for g in range(R // G):
                  gt = gp.tile([128, NCH * CH], F32)
                  # col 6 rows 16-127 are never gathered; clear for finite 0s
                  nc.vector.memset(gt[:, 6 * CH : 7 * CH], 0.0)
                  dest = gt[:, :].rearrange("p (j f) -> p j f", f=CH)
                  nc.gpsimd.dma_gather(
                      dest,
                      data.ap(),
                      IDXG[:, g * NB : (g + 1) * NB],
                      NI,
                      NI,
                      CH,
                      queue_num=g % 4,
                  )
                  for nl in range(G):
                      n = g * G + nl
                      # weighted mask for all 7 chunks of this roi in one op:
                      # wm[p, c, j] = cmask[p, c, j] * WT[p, n*7+c]
                      wm = gwp.tile([128, NCH * NB], F32)
                      wtb = (
                          WT[:, n * NCH : (n + 1) * NCH]
                          .unsqueeze(2)
                          .broadcast_to([128, NCH, NB])
                      )
                      nc.any.tensor_tensor(
                          wm[:, :].rearrange("p (c j) -> p c j", j=NB),
                          cmsk[:, :].rearrange("p (c j) -> p c j", j=NB),
                          wtb,
                          OP.mult,
                      )
                      ps = psp.tile([NB, CH], F32)
                      for c in range(NCH):
                          gv = gt[:, (nl * NCH + c) * CH : (nl * NCH + c + 1) * CH]
                          nc.tensor.matmul(
                              ps[:, :],
                              wm[:, c * NB : (c + 1) * NB],
                              gv,
                              start=(c == 0),
                              stop=(c == NCH - 1),
                          )
                      ob = obp.tile([NB, CH], F32)
                      nc.vector.tensor_copy(ob[:, :], ps[:, :])
                      nc.sync.dma_start(out_r[n : n + 1, :, :], ob[:, :])

    nc.finalize()
    return nc


def host_constants():
    iopw = np.tile((np.arange(NB) % P).astype(np.float32), (R, 1))
    ioph = np.tile((np.arange(NB) // P).astype(np.float32), (R, 1))
    iden = np.eye(R, dtype=np.float32)
    cm = np.zeros((128, NCH * NB), dtype=np.float32)
    for j in range(6):
        for p in range(128):
            cm[p, j * NB + 8 * j + p // TPB] = 1.0
    for p in range(TPB):
        cm[p, 6 * NB + 48] = 1.0
    return {"iota_pw": iopw, "iota_ph": ioph, "identity": iden, "cmasks": cm}


_cache = {}


def _program():
    if "nc" not in _cache:
        _cache["nc"] = build_program()
    return _cache["nc"]


def run(data, rois, offset, **spmd_kwargs):
    data = np.asarray(data, dtype=np.float32)
    rois = np.asarray(rois, dtype=np.float32)
    offset = np.asarray(offset, dtype=np.float32)
    n_rois = rois.shape[0]
    data_t = np.ascontiguousarray(data.transpose(0, 2, 3, 1)).reshape(NPX, CH)
    consts = host_constants()
    in_maps = []
    for c in range(N_CORES):
        sl = slice(c * R, (c + 1) * R)
        m = {
            "data_t": data_t,
            "rois": rois[sl],
            "offs": offset[sl].reshape(R, 2 * NB),
        }
        m.update(consts)
        in_maps.append(m)
    res = run_bass_kernel_spmd(
        _program(), in_maps, core_ids=list(range(N_CORES)), **spmd_kwargs
    )
    outs = np.concatenate([res.results[c]["out"] for c in range(N_CORES)], axis=0)
    out = outs.reshape(n_rois, NB, CH).transpose(0, 2, 1).reshape(n_rois, CH, P, P)
    return np.ascontiguousarray(out), res


def kernel(data, rois, offset):
    out, _ = run(data, rois, offset)
    return out

